# revision 13
# baseline (speedup 1.0000x reference)
"""ContrastiveAttention Trainium2 Bass kernel.

reference:
    scores = einsum('qm,bdm->bqd', query, memory)   # q=128, m=1024, b=128, d=1024
    attn   = softmax(-scores, axis=2)               # softmin over memory slots d
    out    = einsum('bqd,bdm->bqm', attn, memory)

Sharding: batch dim of memory split across 8 cores (16 batches each), query
replicated; each core computes its [16, 128, 1024] output slice independently.

Per-core per-batch plan (all fp32):
  - load memory[b] natural layout  mn[d_part, m]    (8 tiles [128,1024])
  - PE-transpose 64 128x128 blocks -> memT[m_part, d] for mm1
  - mm1: psum = (-qT).T @ memT  = -scores[q, d]     (fp32, K=m, 8 k-tiles, 2 banks)
  - softmin: negmx = -max(psum); attn_u = exp(psum + negmx) with accum_out sums
  - PE-transpose attn -> attnT[d_part, q]
  - mm2: psum = attnT.T @ mn = unnormalized out[q, m']
  - evict fused with 1/sum normalization, DMA store
"""

import os
import sys

sys.path.insert(0, "/opt/trn_rl_repo")

import numpy as np

import concourse.bass as bass
import concourse.mybir as mybir
import concourse.tile as tile
from concourse.bass import ts
from concourse.bass_utils import run_bass_kernel_spmd
from concourse.masks import make_identity

B, D, DM = 128, 1024, 1024
N_CORES = 8
B_LOC = B // N_CORES  # 16
P = 128
KT = DM // P  # 8 k-tiles over m
DT = D // P   # 8 tiles over d
F32 = mybir.dt.float32
AX = mybir.AxisListType
ALU = mybir.AluOpType
ACTF = mybir.ActivationFunctionType


def _body(tc, out, query, memory, b_loc=B_LOC):
    nc = tc.nc
    with (
        tc.tile_pool(name="singles", bufs=1) as singles,
        tc.tile_pool(name="mn", bufs=3) as mn_pool,
        tc.tile_pool(name="mt", bufs=2) as mt_pool,
        tc.tile_pool(name="attn", bufs=2) as attn_pool,
        tc.tile_pool(name="attnT", bufs=1) as attnT_pool,
        tc.tile_pool(name="outp", bufs=2) as out_pool,
        tc.tile_pool(name="stats", bufs=8) as stats,
        tc.tile_pool(name="ptr", bufs=2, space="PSUM") as ptr_pool,
        tc.tile_pool(name="ps", bufs=1, space="PSUM") as ps_pool,
        tc.tile_pool(name="pat", bufs=2, space="PSUM") as pat_pool,
        tc.tile_pool(name="po", bufs=1, space="PSUM") as po_pool,
    ):
        ident = singles.tile([P, P], F32)
        make_identity(nc, ident)

        # Dummy transpose so the PE observes the Pool-engine (identity) tick
        # on its own 1-wait instruction; walrus's fp32 LDW path rejects
        # matmuls carrying >1 semaphore wait. The qT setup uses the `pat`
        # pool so the `ptr` pool slots stay fresh for the first memT batch
        # (a WAR on the setup's ACT reads would add a second wait there).
        warm = pat_pool.tile([P, P], F32, tag="pat")
        nc.tensor.transpose(warm[:, :], ident[:, :], ident[:, :])

        # qTneg[:, kt*128:(kt+1)*128] = -query[:, kt*128:(kt+1)*128].T
        q_sb = singles.tile([P, DM], F32)
        nc.sync.dma_start(out=q_sb[:, :], in_=query[:, :])
        qTneg = singles.tile([P, KT * P], F32)
        for kt in range(KT):
            p_t = pat_pool.tile([P, P], F32, tag="pat")
            nc.tensor.transpose(p_t[:, :], q_sb[:, ts(kt, P)], ident)
            nc.scalar.mul(qTneg[:, ts(kt, P)], p_t[:, :], -1.0)

        mn_tiles = {}
        mt_tiles = {}

        def emit_loads(b):
            t = mn_pool.tile([P, DT, DM], F32, tag="mn")
            mn_tiles[b] = t
            for dt in range(DT):
                nc.sync.dma_start(
                    out=t[:, dt, :], in_=memory[b, dt * P : (dt + 1) * P, :]
                )

        def emit_memT(b):
            # mt[m_local(block mt), mt, d_global] = memory[b, d, mt*128+m_local]
            mn_t = mn_tiles[b]
            mt_t = mt_pool.tile([P, KT, D], F32, tag="mt")
            mt_tiles[b] = mt_t
            for mt in range(KT):
                for g in range(2):  # groups of 4 d-tiles -> one psum bank
                    p_t = ptr_pool.tile([P, 512], F32, tag="ptr")
                    for j in range(4):
                        dt = g * 4 + j
                        nc.tensor.transpose(
                            p_t[:, ts(j, P)], mn_t[:, dt, ts(mt, P)], ident
                        )
                    # All memT evictions on ACT: keeps every PE matmul/transpose
                    # waiting on at most one foreign engine clock.
                    dst = mt_t[:, mt, g * 512 : (g + 1) * 512]
                    nc.scalar.copy(dst, p_t[:, :])

        emit_loads(0)
        emit_memT(0)
        if b_loc > 1:
            emit_loads(1)

        for b in range(b_loc):
            mn_t = mn_tiles[b]
            mt_t = mt_tiles[b]

            # mm1: ps = -scores[q, d]
            ps_t = ps_pool.tile([P, 2, 512], F32, tag="ps")
            for nb in range(2):
                for kt in range(KT):
                    nc.tensor.matmul(
                        ps_t[:, nb, :],
                        lhsT=qTneg[:, ts(kt, P)],
                        rhs=mt_t[:, kt, nb * 512 : (nb + 1) * 512],
                        start=(kt == 0),
                        stop=(kt == KT - 1),
                    )

            # softmin stats: negmx = -max(-scores) = min(scores)
            negmx = stats.tile([P, 1], F32, tag="negmx")
            nc.vector.tensor_reduce(
                out=negmx[:, :], in_=ps_t[:, :, :], axis=AX.XY, op=ALU.max, negate=True
            )
            # ACT-side copy of the DVE-produced bias so the Exp instruction
            # only waits on PE (scores) — ACT deps become same-engine.
            negmx2 = stats.tile([P, 1], F32, tag="negmx2")
            nc.scalar.copy(negmx2[:, :], negmx[:, :])
            attn_t = attn_pool.tile([P, D], F32, tag="attn")
            ssum = stats.tile([P, 2], F32, tag="ssum")
            for nb in range(2):
                nc.scalar.activation(
                    out=attn_t[:, nb * 512 : (nb + 1) * 512],
                    in_=ps_t[:, nb, :],
                    func=ACTF.Exp,
                    bias=negmx2[:, :],
                    scale=1.0,
                    accum_out=ssum[:, nb : nb + 1],
                )
            s1 = stats.tile([P, 1], F32, tag="s1")
            nc.vector.reduce_sum(out=s1[:, :], in_=ssum[:, :], axis=AX.X)
            rcp = stats.tile([P, 1], F32, tag="rcp")
            nc.vector.reciprocal(rcp[:, :], s1[:, :])

            # PE transposes for next batch run here: they cover the softmax
            # latency so mm2 (which needs attnT) doesn't stall the PE.
            if b + 1 < b_loc:
                emit_memT(b + 1)

            # attnT + mm2 interleaved per d-tile
            po_t = po_pool.tile([P, 2, 512], F32, tag="po")
            at_t = attnT_pool.tile([P, DT, P], F32, tag="attnT")
            for dt in range(DT):
                p_at = pat_pool.tile([P, P], F32, tag="pat")
                nc.tensor.transpose(p_at[:, :], attn_t[:, ts(dt, P)], ident)
                nc.vector.tensor_copy(at_t[:, dt, :], p_at[:, :])
                for nb in range(2):
                    nc.tensor.matmul(
                        po_t[:, nb, :],
                        lhsT=at_t[:, dt, :],
                        rhs=mn_t[:, dt, nb * 512 : (nb + 1) * 512],
                        start=(dt == 0),
                        stop=(dt == DT - 1),
                    )

            # evict with fused 1/sum normalization
            o_t = out_pool.tile([P, DM], F32, tag="outp")
            for nb in range(2):
                nc.vector.tensor_scalar_mul(
                    o_t[:, nb * 512 : (nb + 1) * 512], po_t[:, nb, :], rcp[:, :]
                )
            nc.sync.dma_start(out=out[b, :, :], in_=o_t[:, :])

            if b + 2 < b_loc:
                emit_loads(b + 2)


def _spill_waits(nc):
    """Walrus instruction structs encode only 1-2 sem waits (PE matmul/ldw:
    1; DVE tensor-scalar: 2; pseudo-DMA / ctrl: 1-2), and Tile's scheduler
    sometimes attaches more. Rewrite every block, spilling the waits of any
    multi-wait instruction onto preceding single-wait NoOps on the same
    engine — sequencers process instructions in order, so this is
    semantically identical."""
    f = nc.m.functions[0]
    for blk in f.blocks:
        il = blk.instructions
        out = []
        changed = False
        for inst in il:
            si = getattr(inst, "sync_info", None)
            if (
                si is not None
                and si.on_wait
                and len(si.on_wait) > 1
                and not isinstance(inst, mybir.InstNoOp)
            ):
                for w in si.on_wait:
                    nop = mybir.InstNoOp(
                        name=nc.get_next_instruction_name(),
                        ins=[],
                        outs=[],
                        sync_info=mybir.SyncInfo(on_wait=[w], on_update=[]),
                        bass_nofuse=True,
                        engine=inst.engine,
                    )
                    out.append(nop)
                inst.sync_info = mybir.SyncInfo(
                    on_wait=[], on_update=list(si.on_update or [])
                )
                changed = True
            out.append(inst)
        if changed:
            il[:] = out


def _build(b_loc=B_LOC):
    nc = bass.Bass(
        "TRN2", target_bir_lowering=False, debug=False, num_devices=N_CORES
    )
    query = nc.dram_tensor("query", [B, DM], F32, kind="ExternalInput").ap()
    memory = nc.dram_tensor("memory", [b_loc, D, DM], F32, kind="ExternalInput").ap()
    out = nc.dram_tensor("out", [b_loc, B, DM], F32, kind="ExternalOutput").ap()
    with tile.TileContext(nc) as tc:
        _body(tc, out, query, memory, b_loc=b_loc)
    _spill_waits(nc)
    return nc


def _run(query, memory, trace=False, **trace_kwargs):
    query = np.ascontiguousarray(np.asarray(query, dtype=np.float32))
    memory = np.ascontiguousarray(np.asarray(memory, dtype=np.float32))
    assert query.shape == (B, DM) and memory.shape == (B, D, DM)
    nc = _build()
    in_maps = [
        {
            "query": query,
            "memory": np.ascontiguousarray(memory[c * B_LOC : (c + 1) * B_LOC]),
        }
        for c in range(N_CORES)
    ]
    res = run_bass_kernel_spmd(
        nc, in_maps, list(range(N_CORES)), trace=trace, **trace_kwargs
    )
    out = np.concatenate(
        [res.results[c]["out"] for c in range(N_CORES)], axis=0
    )
    return out, res


def kernel(query, memory):
    out, _ = _run(query, memory, trace=False)
    return out


if __name__ == "__main__":
    rng = np.random.default_rng(0)
    q = rng.standard_normal((B, DM), dtype=np.float32)
    m = rng.standard_normal((B, D, DM), dtype=np.float32)
    o = kernel(q, m)
    print("out", o.shape, o.dtype, float(np.abs(o).max()))
